# revision 14
# baseline (speedup 1.0000x reference)
"""Trainium2 Bass kernel for nn_NodeEncoding_72816875537095.

Reference computation:
    scores = x @ W[0] + b[0]                          # [total]
    sp     = scatter(scores, pad_idx) -> [B, 96]      # padded per-graph scores
    num    = einsum('bijk,bk->bij', paths, sp)
    den    = paths.sum(-1) + 1e-8
    out    = num / den                                # [64, 96, 96]

Strategy (data-parallel over B across 8 NeuronCores, 8 graphs/core):
  - Host relayout: per core+graph, paths -> k-major [128, 9216] fp8 tiles
    (k rows 96..127 zero-padded; 0/1 exact in fp8, full-partition DMAs).
  - All 8 graph tiles are SBUF-resident (74KB/partition); every paths DMA
    is issued up front on the sync HWDGE queue so the SDMA engines stream
    continuously; aux inputs + output stores ride the scalar HWDGE queue.
  - Scores on PE: per (graph, d-half), one [128,128] bf16 stationary tile
    of x^T (k on columns) x W-half [128,1] moving -> accumulated PSUM
    [128, 8] = scores in exactly the [k, g] layout the moving operand
    needs.  Rows 96..127 come out zero for free.
  - Main loop: per 128-column chunk of a graph, ONE matmul with the paths
    chunk as fp8 stationary (FWL) and a 3-column moving operand
    [sp_hi, sp_lo, ones] -> PSUM [128, 3] = (num_hi, num_lo, den).
    sp is hi/lo fp8-split (~8 mantissa bits; tolerance is 2e-2).
  - Per-graph PSUM tile [128, 216]; epilogue is 3 wide strided ops:
    rec = ACT Reciprocal(den + eps), num = DVE stt(lo*2^-4 + hi),
    out = DVE num*rec.
  - Output is stored partition-major [128, 576]; host un-permutes.
"""

import sys

if "/opt/trn_rl_repo" not in sys.path:
    sys.path.insert(0, "/opt/trn_rl_repo")

import ml_dtypes
import numpy as np

import concourse.bass as bass  # noqa: F401
import concourse.mybir as mybir
from concourse import bacc, bass_utils
from concourse.tile import TileContext

F32 = mybir.dt.float32
BF16 = mybir.dt.bfloat16
FP8 = mybir.dt.float8e4
AF = mybir.ActivationFunctionType

B = 64
MAX_A = 96
D = 256
N_CORES = 8
G = B // N_CORES            # 8 graphs per core
COLS = MAX_A * MAX_A        # 9216
KP = 128                    # padded contraction rows
CHUNK = 128                 # stationary columns per matmul
CPG = COLS // CHUNK         # 72 chunks per graph
TOT = G * CPG               # 576 chunks per core
EPS = 1e-8
# per-graph paths DMA split (columns): graphs late in the stream arrive in
# smaller pieces so compute can chase the tail.
SPLITS = [1, 1, 1, 1, 1, 1, 1, 2]

_NC_CACHE = {}


def _build():
    if "nc" in _NC_CACHE:
        return _NC_CACHE["nc"]

    nc = bacc.Bacc("TRN2", target_bir_lowering=False, debug=False,
                   num_devices=N_CORES)

    pathsT_d = nc.dram_tensor("pathsT", [G, KP, COLS], FP8,
                              kind="ExternalInput")
    xt_d = nc.dram_tensor("xt", [KP, 2 * G * CHUNK], BF16,
                          kind="ExternalInput")
    wt_d = nc.dram_tensor("wt", [KP, 2], BF16, kind="ExternalInput")
    bmask_d = nc.dram_tensor("bmask", [KP, G], F32, kind="ExternalInput")
    out_d = nc.dram_tensor("out", [CHUNK, TOT], F32, kind="ExternalOutput")

    with TileContext(nc) as tc:
        with (
            tc.tile_pool(name="misc", bufs=1) as misc,
            tc.tile_pool(name="paths", bufs=8) as ppool,
            tc.tile_pool(name="spsum", bufs=1, space="PSUM") as sps,
            tc.tile_pool(name="psum", bufs=4, space="PSUM") as pspool,
            tc.tile_pool(name="epi", bufs=3) as epool,
        ):
            # ---- every input DMA up front; triggers cost ~600ns of engine
            # time each, so split them across BOTH HWDGE engines.  The
            # scores pipeline needs xt/wt first. ----
            xt = misc.tile([KP, 2 * G * CHUNK], BF16)
            nc.sync.dma_start(out=xt[:], in_=xt_d[:])
            wt = misc.tile([KP, 2], BF16)
            nc.scalar.dma_start(out=wt[:], in_=wt_d[:])
            bm = misc.tile([KP, G], F32)
            nc.scalar.dma_start(out=bm[:], in_=bmask_d[:])

            st = {}
            for g in range(G):
                st[g] = ppool.tile([KP, COLS], FP8, tag="st", name=f"st{g}")
            for g in [0, 2, 4, 6, 1, 3, 5, 7]:
                eng = nc.sync if g % 2 == 0 else nc.scalar
                n = SPLITS[g]
                w = COLS // n
                for s in range(n):
                    eng.dma_start(out=st[g][:, s * w:(s + 1) * w],
                                  in_=pathsT_d[g][:, s * w:(s + 1) * w])

            # ---- node scores on PE -> PSUM [128, G] (k-major) ----
            sp_ps = sps.tile([KP, G], F32, tag="sc")
            for g in range(G):
                for h in range(2):
                    nc.tensor.matmul(
                        sp_ps[:, g:g + 1],
                        lhsT=xt[:, (h * G + g) * CHUNK:
                                (h * G + g + 1) * CHUNK],
                        rhs=wt[:, h:h + 1],
                        start=(h == 0), stop=(h == 1))

            # w_all columns per graph g: [3g..3g+3) = [sp_hi, sp_lo*16, one]
            w_sp = misc.tile([KP, G], F32)
            nc.vector.tensor_tensor(out=w_sp[:], in0=sp_ps[:], in1=bm[:],
                                    op=mybir.AluOpType.add)
            w_hi = misc.tile([KP, G], FP8)
            nc.vector.tensor_copy(w_hi[:], w_sp[:])
            r1 = misc.tile([KP, G], F32)
            nc.vector.tensor_tensor(out=r1[:], in0=w_sp[:], in1=w_hi[:],
                                    op=mybir.AluOpType.subtract)
            w_all = misc.tile([KP, 3 * G], FP8)
            nc.vector.memset(w_all[:, 2:3 * G:3], 1.0)
            nc.vector.tensor_copy(w_all[:, 0:3 * G:3], w_hi[:])
            nc.vector.tensor_scalar_mul(out=w_all[:, 1:3 * G:3], in0=r1[:],
                                        scalar1=16.0)

            out_sb = misc.tile([CHUNK, TOT], F32)

            # ---- main loop: one matmul per 128-column chunk ----
            # graph 7 is processed as two half-tiles so the tail (last DMA
            # -> last MM -> epilogue -> store) is as short as possible.
            pieces = [(g, 0, CPG) for g in range(G - 1)]
            pieces += [(G - 1, 0, CPG // 2), (G - 1, CPG // 2, CPG)]
            for g, c0, c1 in pieces:
                w = c1 - c0
                ps = pspool.tile([CHUNK, 3 * w], F32, tag="ps")
                for cl in range(c0, c1):
                    r = cl - c0
                    nc.tensor.matmul(
                        ps[:, 3 * r:3 * r + 3],
                        lhsT=st[g][:, CHUNK * cl:CHUNK * (cl + 1)],
                        rhs=w_all[:, 3 * g:3 * g + 3],
                        start=True, stop=True)
                # epilogue: out = (hi + lo/16) * 1/(den + eps)
                den = epool.tile([CHUNK, CPG], F32, tag="den")
                nc.scalar.activation(out=den[:, :w], in_=ps[:, 2:3 * w:3],
                                     func=AF.Copy, bias=EPS)
                rec = epool.tile([CHUNK, CPG], F32, tag="rec")
                nc.vector.reciprocal(out=rec[:, :w], in_=den[:, :w])
                hi = epool.tile([CHUNK, CPG], F32, tag="hi")
                nc.scalar.activation(out=hi[:, :w], in_=ps[:, 0:3 * w:3],
                                     func=AF.Copy)
                numt = epool.tile([CHUNK, CPG], F32, tag="numt")
                nc.vector.scalar_tensor_tensor(
                    out=numt[:, :w], in0=ps[:, 1:3 * w:3], scalar=0.0625,
                    in1=hi[:, :w],
                    op0=mybir.AluOpType.mult, op1=mybir.AluOpType.add)
                nc.vector.tensor_tensor(
                    out=out_sb[:, CPG * g + c0:CPG * g + c1],
                    in0=numt[:, :w], in1=rec[:, :w],
                    op=mybir.AluOpType.mult)
                if g == 2 and c1 == CPG:
                    nc.scalar.dma_start(out=out_d[:, :3 * CPG],
                                        in_=out_sb[:, :3 * CPG])
                if g == 5 and c1 == CPG:
                    nc.scalar.dma_start(out=out_d[:, 3 * CPG:6 * CPG],
                                        in_=out_sb[:, 3 * CPG:6 * CPG])
            nc.sync.dma_start(out=out_d[:, 6 * CPG:],
                              in_=out_sb[:, 6 * CPG:])

    nc.compile()
    _NC_CACHE["nc"] = nc
    return nc


def _host_prep(x, W, b, paths, pad_idx):
    x = np.ascontiguousarray(np.asarray(x, dtype=np.float32))
    W = np.asarray(W, dtype=np.float32)
    b = np.asarray(b, dtype=np.float32)
    pad_idx = np.asarray(pad_idx)

    # scatter x into padded [B*MAX_A, D] layout, mark valid slots
    xsc = np.zeros((B * MAX_A, D), dtype=np.float32)
    xsc[pad_idx] = x
    valid = np.zeros((B * MAX_A,), dtype=np.float32)
    valid[pad_idx] = 1.0
    bmask_full = (b[0] * valid).reshape(B, MAX_A)

    paths_f8 = np.asarray(paths).astype(ml_dtypes.float8_e4m3)
    wt_all = np.zeros((KP, 2), dtype=ml_dtypes.bfloat16)
    wt_all[:, 0] = W[0, :KP]
    wt_all[:, 1] = W[0, KP:]

    in_maps = []
    for core in range(N_CORES):
        g0 = core * G
        pc = paths_f8[g0:g0 + G]  # [G, 96, 96, 96]
        pathsT = np.zeros((G, KP, COLS), dtype=ml_dtypes.float8_e4m3)
        pathsT[:, :MAX_A, :] = pc.transpose(0, 3, 1, 2).reshape(
            G, MAX_A, COLS)
        # xt[d, h*1024 + g*128 + k] = x[g0+g, k, h*128 + d]; zero k >= 96
        xc = xsc[g0 * MAX_A:(g0 + G) * MAX_A].reshape(G, MAX_A, D)
        xthw = np.pad(xc.transpose(2, 0, 1),         # [D, G, 128]
                      ((0, 0), (0, 0), (0, KP - MAX_A)))
        xt = np.zeros((KP, 2 * G * CHUNK), dtype=ml_dtypes.bfloat16)
        xt[:, :G * CHUNK] = xthw[:KP].reshape(KP, G * CHUNK)
        xt[:, G * CHUNK:] = xthw[KP:].reshape(KP, G * CHUNK)
        bmask = np.zeros((KP, G), dtype=np.float32)
        bmask[:MAX_A, :] = bmask_full[g0:g0 + G].T
        in_maps.append({
            "pathsT": pathsT,
            "xt": xt,
            "wt": wt_all,
            "bmask": bmask,
        })
    return in_maps


LAST_RESULTS = None


def kernel(x, W, b, paths, pad_idx, _trace=False):
    global LAST_RESULTS
    nc = _build()
    in_maps = _host_prep(x, W, b, paths, pad_idx)
    res = bass_utils.run_bass_kernel_spmd(
        nc, in_maps, core_ids=list(range(N_CORES)), trace=_trace)
    LAST_RESULTS = res

    out = np.empty((B, MAX_A, MAX_A), dtype=np.float32)
    for core in range(N_CORES):
        oc = res.results[core]["out"]  # [128, 576] partition-major
        out[core * G:(core + 1) * G] = oc.T.reshape(G, MAX_A, MAX_A)
    return out


# revision 15
# speedup vs baseline: 1.0741x; 1.0741x over previous
"""Trainium2 Bass kernel for nn_NodeEncoding_72816875537095.

Reference computation:
    scores = x @ W[0] + b[0]                          # [total]
    sp     = scatter(scores, pad_idx) -> [B, 96]      # padded per-graph scores
    num    = einsum('bijk,bk->bij', paths, sp)
    den    = paths.sum(-1) + 1e-8
    out    = num / den                                # [64, 96, 96]

Strategy (data-parallel over B across 8 NeuronCores, 8 graphs/core):
  - Host relayout: per core+graph, paths -> k-major [128, 9216] fp8 tiles
    (k rows 96..127 zero-padded; 0/1 exact in fp8, full-partition DMAs).
  - All 8 graph tiles are SBUF-resident (74KB/partition); every paths DMA
    is issued up front on the sync HWDGE queue so the SDMA engines stream
    continuously; aux inputs + output stores ride the scalar HWDGE queue.
  - Scores on PE: per (graph, d-half), one [128,128] bf16 stationary tile
    of x^T (k on columns) x W-half [128,1] moving -> accumulated PSUM
    [128, 8] = scores in exactly the [k, g] layout the moving operand
    needs.  Rows 96..127 come out zero for free.
  - Main loop: per 128-column chunk of a graph, ONE matmul with the paths
    chunk as fp8 stationary (FWL) and a 3-column moving operand
    [sp_hi, sp_lo, ones] -> PSUM [128, 3] = (num_hi, num_lo, den).
    sp is hi/lo fp8-split (~8 mantissa bits; tolerance is 2e-2).
  - Per-graph PSUM tile [128, 216]; epilogue is 3 wide strided ops:
    rec = ACT Reciprocal(den + eps), num = DVE stt(lo*2^-4 + hi),
    out = DVE num*rec.
  - Output is stored partition-major [128, 576]; host un-permutes.
"""

import sys

if "/opt/trn_rl_repo" not in sys.path:
    sys.path.insert(0, "/opt/trn_rl_repo")

import ml_dtypes
import numpy as np

import concourse.bass as bass  # noqa: F401
import concourse.mybir as mybir
from concourse import bacc, bass_utils
from concourse.tile import TileContext

F32 = mybir.dt.float32
BF16 = mybir.dt.bfloat16
FP8 = mybir.dt.float8e4
AF = mybir.ActivationFunctionType

B = 64
MAX_A = 96
D = 256
N_CORES = 8
G = B // N_CORES            # 8 graphs per core
COLS = MAX_A * MAX_A        # 9216
KP = 128                    # padded contraction rows
CHUNK = 128                 # stationary columns per matmul
CPG = COLS // CHUNK         # 72 chunks per graph
TOT = G * CPG               # 576 chunks per core
EPS = 1e-8
# per-graph paths DMA split (columns): graphs late in the stream arrive in
# smaller pieces so compute can chase the tail.
SPLITS = [1, 1, 1, 1, 1, 1, 1, 2]

_NC_CACHE = {}


def _build():
    if "nc" in _NC_CACHE:
        return _NC_CACHE["nc"]

    nc = bacc.Bacc("TRN2", target_bir_lowering=False, debug=False,
                   num_devices=N_CORES)

    pathsT_d = nc.dram_tensor("pathsT", [G, KP, COLS], FP8,
                              kind="ExternalInput")
    xt_d = nc.dram_tensor("xt", [KP, 2 * G * CHUNK], BF16,
                          kind="ExternalInput")
    wt_d = nc.dram_tensor("wt", [KP, 2], BF16, kind="ExternalInput")
    bmask_d = nc.dram_tensor("bmask", [KP, G], F32, kind="ExternalInput")
    out_d = nc.dram_tensor("out", [CHUNK, TOT], F32, kind="ExternalOutput")

    with TileContext(nc) as tc:
        with (
            tc.tile_pool(name="misc", bufs=1) as misc,
            tc.tile_pool(name="paths", bufs=8) as ppool,
            tc.tile_pool(name="spsum", bufs=1, space="PSUM") as sps,
            tc.tile_pool(name="psum", bufs=4, space="PSUM") as pspool,
            tc.tile_pool(name="epi", bufs=3) as epool,
        ):
            # ---- every input DMA up front.  All paths tiles go on ONE
            # queue (sync) so graphs complete sequentially and the PE can
            # chase them one at a time; tiny aux rides the scalar queue.
            # xt leads on sync: the scores pipeline gates everything. ----
            xt = misc.tile([KP, 2 * G * CHUNK], BF16)
            nc.sync.dma_start(out=xt[:], in_=xt_d[:])
            wt = misc.tile([KP, 2], BF16)
            nc.scalar.dma_start(out=wt[:], in_=wt_d[:])
            bm = misc.tile([KP, G], F32)
            nc.scalar.dma_start(out=bm[:], in_=bmask_d[:])

            st = {}
            for g in range(G):
                st[g] = ppool.tile([KP, COLS], FP8, tag="st", name=f"st{g}")
                n = SPLITS[g]
                w = COLS // n
                for s in range(n):
                    nc.sync.dma_start(out=st[g][:, s * w:(s + 1) * w],
                                      in_=pathsT_d[g][:, s * w:(s + 1) * w])

            # ---- node scores on PE -> PSUM [128, G] (k-major) ----
            sp_ps = sps.tile([KP, G], F32, tag="sc")
            for g in range(G):
                for h in range(2):
                    nc.tensor.matmul(
                        sp_ps[:, g:g + 1],
                        lhsT=xt[:, (h * G + g) * CHUNK:
                                (h * G + g + 1) * CHUNK],
                        rhs=wt[:, h:h + 1],
                        start=(h == 0), stop=(h == 1))

            # w_all columns per graph g: [3g..3g+3) = [sp_hi, sp_lo*16, one]
            w_sp = misc.tile([KP, G], F32)
            nc.vector.tensor_tensor(out=w_sp[:], in0=sp_ps[:], in1=bm[:],
                                    op=mybir.AluOpType.add)
            w_hi = misc.tile([KP, G], FP8)
            nc.vector.tensor_copy(w_hi[:], w_sp[:])
            r1 = misc.tile([KP, G], F32)
            nc.vector.tensor_tensor(out=r1[:], in0=w_sp[:], in1=w_hi[:],
                                    op=mybir.AluOpType.subtract)
            w_all = misc.tile([KP, 3 * G], FP8)
            nc.vector.memset(w_all[:, 2:3 * G:3], 1.0)
            nc.vector.tensor_copy(w_all[:, 0:3 * G:3], w_hi[:])
            nc.vector.tensor_scalar_mul(out=w_all[:, 1:3 * G:3], in0=r1[:],
                                        scalar1=16.0)

            out_sb = misc.tile([CHUNK, TOT], F32)

            # ---- main loop: one matmul per 128-column chunk ----
            # graph 7 is processed as two half-tiles so the tail (last DMA
            # -> last MM -> epilogue -> store) is as short as possible.
            pieces = [(g, 0, CPG) for g in range(G - 1)]
            pieces += [(G - 1, 0, CPG // 2), (G - 1, CPG // 2, CPG)]
            for g, c0, c1 in pieces:
                w = c1 - c0
                ps = pspool.tile([CHUNK, 3 * w], F32, tag="ps")
                for cl in range(c0, c1):
                    r = cl - c0
                    nc.tensor.matmul(
                        ps[:, 3 * r:3 * r + 3],
                        lhsT=st[g][:, CHUNK * cl:CHUNK * (cl + 1)],
                        rhs=w_all[:, 3 * g:3 * g + 3],
                        start=True, stop=True)
                # epilogue: out = (hi + lo/16) * 1/(den + eps)
                den = epool.tile([CHUNK, CPG], F32, tag="den")
                nc.scalar.activation(out=den[:, :w], in_=ps[:, 2:3 * w:3],
                                     func=AF.Copy, bias=EPS)
                rec = epool.tile([CHUNK, CPG], F32, tag="rec")
                nc.vector.reciprocal(out=rec[:, :w], in_=den[:, :w])
                hi = epool.tile([CHUNK, CPG], F32, tag="hi")
                nc.scalar.activation(out=hi[:, :w], in_=ps[:, 0:3 * w:3],
                                     func=AF.Copy)
                numt = epool.tile([CHUNK, CPG], F32, tag="numt")
                nc.vector.scalar_tensor_tensor(
                    out=numt[:, :w], in0=ps[:, 1:3 * w:3], scalar=0.0625,
                    in1=hi[:, :w],
                    op0=mybir.AluOpType.mult, op1=mybir.AluOpType.add)
                nc.vector.tensor_tensor(
                    out=out_sb[:, CPG * g + c0:CPG * g + c1],
                    in0=numt[:, :w], in1=rec[:, :w],
                    op=mybir.AluOpType.mult)
                if g == 2 and c1 == CPG:
                    nc.scalar.dma_start(out=out_d[:, :3 * CPG],
                                        in_=out_sb[:, :3 * CPG])
                if g == 5 and c1 == CPG:
                    nc.scalar.dma_start(out=out_d[:, 3 * CPG:6 * CPG],
                                        in_=out_sb[:, 3 * CPG:6 * CPG])
            nc.sync.dma_start(out=out_d[:, 6 * CPG:],
                              in_=out_sb[:, 6 * CPG:])

    nc.compile()
    _NC_CACHE["nc"] = nc
    return nc


def _host_prep(x, W, b, paths, pad_idx):
    x = np.ascontiguousarray(np.asarray(x, dtype=np.float32))
    W = np.asarray(W, dtype=np.float32)
    b = np.asarray(b, dtype=np.float32)
    pad_idx = np.asarray(pad_idx)

    # scatter x into padded [B*MAX_A, D] layout, mark valid slots
    xsc = np.zeros((B * MAX_A, D), dtype=np.float32)
    xsc[pad_idx] = x
    valid = np.zeros((B * MAX_A,), dtype=np.float32)
    valid[pad_idx] = 1.0
    bmask_full = (b[0] * valid).reshape(B, MAX_A)

    paths_f8 = np.asarray(paths).astype(ml_dtypes.float8_e4m3)
    wt_all = np.zeros((KP, 2), dtype=ml_dtypes.bfloat16)
    wt_all[:, 0] = W[0, :KP]
    wt_all[:, 1] = W[0, KP:]

    in_maps = []
    for core in range(N_CORES):
        g0 = core * G
        pc = paths_f8[g0:g0 + G]  # [G, 96, 96, 96]
        pathsT = np.zeros((G, KP, COLS), dtype=ml_dtypes.float8_e4m3)
        pathsT[:, :MAX_A, :] = pc.transpose(0, 3, 1, 2).reshape(
            G, MAX_A, COLS)
        # xt[d, h*1024 + g*128 + k] = x[g0+g, k, h*128 + d]; zero k >= 96
        xc = xsc[g0 * MAX_A:(g0 + G) * MAX_A].reshape(G, MAX_A, D)
        xthw = np.pad(xc.transpose(2, 0, 1),         # [D, G, 128]
                      ((0, 0), (0, 0), (0, KP - MAX_A)))
        xt = np.zeros((KP, 2 * G * CHUNK), dtype=ml_dtypes.bfloat16)
        xt[:, :G * CHUNK] = xthw[:KP].reshape(KP, G * CHUNK)
        xt[:, G * CHUNK:] = xthw[KP:].reshape(KP, G * CHUNK)
        bmask = np.zeros((KP, G), dtype=np.float32)
        bmask[:MAX_A, :] = bmask_full[g0:g0 + G].T
        in_maps.append({
            "pathsT": pathsT,
            "xt": xt,
            "wt": wt_all,
            "bmask": bmask,
        })
    return in_maps


LAST_RESULTS = None


def kernel(x, W, b, paths, pad_idx, _trace=False):
    global LAST_RESULTS
    nc = _build()
    in_maps = _host_prep(x, W, b, paths, pad_idx)
    res = bass_utils.run_bass_kernel_spmd(
        nc, in_maps, core_ids=list(range(N_CORES)), trace=_trace)
    LAST_RESULTS = res

    out = np.empty((B, MAX_A, MAX_A), dtype=np.float32)
    for core in range(N_CORES):
        oc = res.results[core]["out"]  # [128, 576] partition-major
        out[core * G:(core + 1) * G] = oc.T.reshape(G, MAX_A, MAX_A)
    return out
